# revision 23
# baseline (speedup 1.0000x reference)
"""BalancedMoE (B=8192, D=2048, E=8, top-2) on 8 Trainium2 NeuronCores.

Strategy: expert-parallel with host-side sparse dispatch.
  - Host computes gate logits / top-2 routing / softmax gates (tiny:
    8192x2048 @ 2048x8), gathers each expert's tokens, and packs them
    into the exact SBUF tile layout so every device DMA is one
    contiguous run per partition.
  - Core e runs a dense [C, D] x [D, D] matmul for expert e only
    (top-2 of 8 experts => 4x less FLOPs than the dense reference),
    with the expert weight matrix stationary in SBUF.
  - Host scatters the per-expert outputs back and combines with the
    gate weights.

Per-core Bass kernel: outT[o, t] = sum_d W_e[o, d] * toks[t, d] + b_e[o]
  lhsT = W_e^T tiles (stationary), rhs = toksT tiles (moving).

v4 (trace-driven, from the fp32r baseline @298us):
  - bf16 operands + outputs: halves every DMA stream; matmul column
    rate is unchanged (1 col/cycle for fp32r>=256col and bf16) but
    bf16 LDWEIGHTS gets FWL (~97ns vs 191ns) so the per-pair overhead
    drops from ~14ns to ~3ns (-14us). Accuracy ~2e-3 vs the 2e-2 gate.
  - Host pre-tiles tokens into [P, KT*C] (per-n-tile SBUF layout):
    HWDGE descriptor generation is ~7ns/DRAM-run, so the old strided
    4-k chunks cost 3.5us each on the SP queue before any data moved —
    that serialization was the real start gate and the n0-sweep stall.
    Contiguous chunks generate in ~0.8us.
  - One output DMA per m row-block ([128, C] accumulated in SBUF)
    instead of 80 per-(n,m) DMAs: fewer SP triggers, fewer semaphores
    (the end-of-NEFF barrier walks every semaphore: ~57 EVENT_SEMs per
    engine = ~7us of epilogue in the baseline).
"""

import os

import numpy as np

P = 128
B = 8192
D_LAT = 1024
D_EMB = 1024
D = D_LAT + D_EMB  # 2048
E = 8
TOPK = 2
N_CORES = 8
KT = D // P
MT = D // P


# ----------------------------------------------------------------- device ---

_cache = {}


def _ntff_shim():
    """Register the axon NTFF profile hook that the boot skips when
    antenv.axon_hooks is missing (so BASS_TRACE=1 yields exec_time_ns)."""
    import sys
    import types

    if "antenv.axon_hooks" in sys.modules:
        return
    holder = [None]
    mod = types.ModuleType("antenv.axon_hooks")
    mod.set_axon_ntff_profile_hook = lambda h: holder.__setitem__(0, h)
    mod.get_axon_ntff_profile_hook = lambda: holder[0]
    sys.modules["antenv.axon_hooks"] = mod
    try:
        import antenv

        antenv.axon_hooks = mod
        from trn_agent_boot.trn_boot import _ntff_profile_via_ctypes

        mod.set_axon_ntff_profile_hook(
            _ntff_profile_via_ctypes("/opt/axon/libaxon_pjrt.so")
        )
    except Exception:
        pass


def _n_tiles(C):
    """Split C into moving-operand tiles of width 256..512 (float32r needs
    >=256 columns per matmul for full PE rate; PSUM caps a tile at 512).
    Large tiles first so the fetch-bound start phase has dense PE work."""
    assert C >= 512
    k = (C - 256) // 512 if C % 512 else C // 512
    rem = C - 512 * k
    sizes = [512] * k
    if rem == 0:
        pass
    elif rem <= 512:
        sizes.append(rem)
    else:  # 513..767: two tiles, both >= 256
        sizes.extend([rem - 256, 256])
    # smallest tile FIRST (cheap first sweep: ~0.26MB first chunk and the
    # DVFS ramp taxes 256-col matmuls at half the 512-col cost), then the
    # 512s (weight window stays wide), second-smallest last (small tail).
    # Full-ascending is a measured loss: it compresses the weight-demand
    # window to ~28us and oversubscribes HBM.
    if len(sizes) >= 3:
        sizes = [sizes[-1]] + sizes[:-2] + [sizes[-2]]
    return sizes


def _build(C, dt_name):
    import concourse.mybir as mybir
    from concourse import bacc
    from concourse.bass import ds
    from concourse.tile import TileContext

    dt_in = getattr(mybir.dt, dt_name)
    dt_out = mybir.dt.float32 if dt_name == "float32r" else dt_in
    n_sizes = _n_tiles(C)
    nc = bacc.Bacc(
        "TRN2", target_bir_lowering=False, debug=False, num_devices=N_CORES
    )
    # wp[m, ki, ko, o] = W_e[m*128 + o, ko*128 + ki] — per-m chunks are
    # contiguous (one run per partition) so descriptor gen is cheap.
    wp = nc.dram_tensor("wp", [MT, P, KT, P], dt_in, kind="ExternalInput")
    # toksP[ki, KT*off_n + ko*n_sz + j] = inp[token (off_n + j), ko*128 + ki]
    # — each n-tile (and any k-chunk of it) is one contiguous run per
    # partition, matching the SBUF tile layout exactly.
    toksP = nc.dram_tensor("toksP", [P, KT * C], dt_in, kind="ExternalInput")
    # biasP[mi, mo] = b_e[mo*128 + mi]
    biasP = nc.dram_tensor("biasP", [P, MT], mybir.dt.float32, kind="ExternalInput")
    outT = nc.dram_tensor("outT", [D, C], dt_out, kind="ExternalOutput")

    o_r = outT.ap().rearrange("(mo mi) n -> mi mo n", mi=P)

    with TileContext(nc) as tc:
        with (
            tc.tile_pool(name="w", bufs=1) as w_pool,
            tc.tile_pool(name="tok", bufs=3) as tok_pool,
            tc.tile_pool(name="out", bufs=1) as out_pool,
            tc.tile_pool(name="bias", bufs=1) as b_pool,
            tc.tile_pool(name="ps", bufs=8, space="PSUM") as ps_pool,
        ):
            bias_tile = b_pool.tile([P, MT], mybir.dt.float32)
            tok_tiles = {}
            n_offs = []
            off = 0
            for sz in n_sizes:
                n_offs.append(off)
                off += sz

            def load_toks(n, chunked=False):
                n_sz = n_sizes[n]
                base = KT * n_offs[n]
                t_full = tok_pool.tile(
                    [P, KT * 512], dt_in, tag="tok", name=f"t{n}"
                )
                t_tile = t_full[:, : KT * n_sz]
                if chunked:
                    # 4-k-slice chunks so the first matmuls only wait for
                    # the slices they read
                    step = 4 * n_sz
                    for g in range(0, KT * n_sz, step):
                        nc.sync.dma_start(
                            t_tile[:, ds(g, step)],
                            toksP.ap()[:, ds(base + g, step)],
                        )
                else:
                    # prefetch tiles ride the Act queue BEHIND the weight
                    # chunks: demand-ordered issue keeps the 2MB-per-tile
                    # prefetches out of the BW-crunched startup window
                    # (they stole HBM from w1/w2 and the n0 chunks: ~4us
                    # of early PE stalls)
                    nc.scalar.dma_start(t_tile, toksP.ap()[:, ds(base, KT * n_sz)])
                tok_tiles[n] = t_tile

            w_tiles = [None] * MT

            def load_w(m):
                w_t = w_pool.tile([P, KT, P], dt_in, tag=f"w{m}")
                # weights ride the Activation-HWDGE queues; tokens/outputs
                # ride SP-HWDGE, so the streams don't interleave in one queue
                nc.scalar.dma_start(w_t[:], wp.ap()[m])
                w_tiles[m] = w_t

            # per-m output accumulators: all 5 n-tile drains for row-block m
            # land here, then one [128, C] DMA writes the block out
            out_tiles = [
                out_pool.tile([P, C], dt_out, tag=f"o{m}", name=f"o{m}")
                for m in range(MT)
            ]

            # issue order ~= consumption order: w0 on the Act queue; token
            # tile 0 (k-chunked) leads the SP queue, bias + deep token
            # prefetch behind it; weight chunks stream behind w0.
            # (No PE warmup: the DVFS ramp is ~4.5us of wall time from
            # stream start regardless, and warmup matmuls gated on any DMA
            # just delay the real stream — measured net loss.)
            load_w(0)
            load_toks(0, chunked=True)
            nc.sync.dma_start(bias_tile[:], biasP.ap())
            for m in range(1, MT):
                load_w(m)
            load_toks(1)

            # out halves: the left half DMAs as soon as its last drain lands
            # (during a mid sweep), so the tail only moves the right half
            if len(n_sizes) >= 3:
                n_half = len(n_sizes) - 2
                split_at = n_offs[n_half] + n_sizes[n_half]
            else:
                n_half = -1
                split_at = 0

            for n, n_sz in enumerate(n_sizes):
                if n + 2 < len(n_sizes):
                    load_toks(n + 2)
                t_tile = tok_tiles.pop(n)
                n_off = n_offs[n]
                for m in range(MT):
                    ps_full = ps_pool.tile([P, 512], mybir.dt.float32, tag="ps")
                    ps = ps_full[:, :n_sz]
                    for k in range(KT):
                        nc.tensor.matmul(
                            ps,
                            w_tiles[m][:, k, :],
                            t_tile[:, ds(k * n_sz, n_sz)],
                            start=(k == 0),
                            stop=(k == KT - 1),
                        )
                    nc.vector.tensor_scalar_add(
                        out_tiles[m][:, ds(n_off, n_sz)],
                        ps,
                        bias_tile[:, m : m + 1],
                    )
                    if n == n_half:
                        nc.sync.dma_start(
                            o_r[:, m, ds(0, split_at)],
                            out_tiles[m][:, ds(0, split_at)],
                        )
                    elif n == len(n_sizes) - 1:
                        nc.sync.dma_start(
                            o_r[:, m, ds(split_at, C - split_at)],
                            out_tiles[m][:, ds(split_at, C - split_at)],
                        )
    nc.compile()
    return nc


def _get_program(C, dt_name):
    key = (C, dt_name)
    if key not in _cache:
        _cache[key] = _build(C, dt_name)
    return _cache[key]


# ------------------------------------------------------------------- host ---


def kernel(x, y, W_experts, b_experts, W_gate, b_gate):
    x = np.asarray(x, dtype=np.float32)
    y = np.asarray(y, dtype=np.float32)
    W_experts = np.asarray(W_experts, dtype=np.float32)
    b_experts = np.asarray(b_experts, dtype=np.float32)
    W_gate = np.asarray(W_gate, dtype=np.float32)
    b_gate = np.asarray(b_gate, dtype=np.float32)

    inp = np.concatenate([x, y], axis=1)  # [B, D]

    # ---- routing (host) ----
    logits = inp.astype(np.float64) @ W_gate.T.astype(np.float64) + b_gate
    order = np.argsort(-logits, axis=1, kind="stable")
    top2 = order[:, :TOPK]  # [B, 2]
    v = np.take_along_axis(logits, top2, axis=1)
    v = v - v.max(axis=1, keepdims=True)
    ev = np.exp(v)
    g = (ev / ev.sum(axis=1, keepdims=True)).astype(np.float32)  # [B, 2]

    counts = np.bincount(top2.ravel(), minlength=E)
    C = max(512, int(counts.max()))

    idx_list = []
    wgt_list = []
    for e in range(E):
        m0 = top2[:, 0] == e
        m1 = top2[:, 1] == e
        idx_e = np.concatenate([np.nonzero(m0)[0], np.nonzero(m1)[0]])
        w_e = np.concatenate([g[m0, 0], g[m1, 1]])
        idx_list.append(idx_e)
        wgt_list.append(w_e)

    dt_name = os.environ.get("MOE_DT", "bfloat16")
    if dt_name == "bfloat16":
        import ml_dtypes

        np_in_dt = np.dtype(ml_dtypes.bfloat16)
    else:
        np_in_dt = np.dtype(np.float32)

    n_sizes = _n_tiles(C)
    inpT = np.ascontiguousarray(inp.T)  # [D, B]
    in_maps = []
    for e in range(E):
        toksT = np.zeros((D, C), dtype=np_in_dt)
        toksT[:, : len(idx_list[e])] = inpT[:, idx_list[e]].astype(np_in_dt)
        # toksP[ki, KT*off + ko*sz + j] = toksT[ko*128 + ki, off + j]
        toksP = np.empty((P, KT * C), dtype=np_in_dt)
        off = 0
        for sz in n_sizes:
            blk = toksT[:, off : off + sz].reshape(KT, P, sz)
            toksP[:, KT * off : KT * (off + sz)] = blk.transpose(1, 0, 2).reshape(
                P, KT * sz
            )
            off += sz
        # wp[m, ki, ko, o] = W_e[m*128 + o, ko*128 + ki]
        wp = np.ascontiguousarray(
            W_experts[e].reshape(MT, P, KT, P).transpose(0, 3, 2, 1).astype(np_in_dt)
        )
        biasP = np.ascontiguousarray(b_experts[e].reshape(MT, P).T)
        in_maps.append({"wp": wp, "toksP": toksP, "biasP": biasP})

    # ---- device ----
    if os.environ.get("BASS_TRACE"):
        _ntff_shim()
    from concourse.bass_utils import run_bass_kernel_spmd

    nc = _get_program(C, dt_name)
    res = None
    for attempt in range(3):
        try:
            res = run_bass_kernel_spmd(nc, in_maps, core_ids=list(range(N_CORES)))
            break
        except Exception:
            # the axon-tunneled device occasionally reports a transient
            # NRT_EXEC_UNIT_UNRECOVERABLE; it recovers after a short wait
            if attempt == 2:
                raise
            import time

            time.sleep(20 * (attempt + 1))
            try:
                import jax

                jax.clear_caches()
            except Exception:
                pass
    globals()["_last_res"] = res
    if res.exec_time_ns is not None:
        print(f"HW exec time: {res.exec_time_ns} ns")

    # ---- combine (host) ----
    fused = np.zeros((B, D), dtype=np.float32)
    for e in range(E):
        n_e = len(idx_list[e])
        if n_e == 0:
            continue
        out_rows = np.asarray(
            res.results[e]["outT"][:, :n_e], dtype=np.float32
        ).T  # [n_e, D]
        fused[idx_list[e]] += out_rows * wgt_list[e][:, None]
    return fused


# revision 25
# speedup vs baseline: 1.1815x; 1.1815x over previous
"""BalancedMoE (B=8192, D=2048, E=8, top-2) on 8 Trainium2 NeuronCores.

Strategy: expert-parallel with host-side sparse dispatch.
  - Host computes gate logits / top-2 routing / softmax gates (tiny:
    8192x2048 @ 2048x8), gathers each expert's tokens, and packs them
    into the exact SBUF tile layout so every device DMA is one
    contiguous run per partition.
  - Core e runs a dense [C, D] x [D, D] matmul for expert e only
    (top-2 of 8 experts => 4x less FLOPs than the dense reference),
    with the expert weight matrix stationary in SBUF.
  - Host scatters the per-expert outputs back and combines with the
    gate weights.

Per-core Bass kernel: outT[o, t] = sum_d W_e[o, d] * toks[t, d] + b_e[o]
  lhsT = W_e^T tiles (stationary), rhs = toksT tiles (moving).

v4 (trace-driven, from the fp32r baseline @298us):
  - bf16 operands + outputs: halves every DMA stream; matmul column
    rate is unchanged (1 col/cycle for fp32r>=256col and bf16) but
    bf16 LDWEIGHTS gets FWL (~97ns vs 191ns) so the per-pair overhead
    drops from ~14ns to ~3ns (-14us). Accuracy ~2e-3 vs the 2e-2 gate.
  - Host pre-tiles tokens into [P, KT*C] (per-n-tile SBUF layout):
    HWDGE descriptor generation is ~7ns/DRAM-run, so the old strided
    4-k chunks cost 3.5us each on the SP queue before any data moved —
    that serialization was the real start gate and the n0-sweep stall.
    Contiguous chunks generate in ~0.8us.
  - One output DMA per m row-block ([128, C] accumulated in SBUF)
    instead of 80 per-(n,m) DMAs: fewer SP triggers, fewer semaphores
    (the end-of-NEFF barrier walks every semaphore: ~57 EVENT_SEMs per
    engine = ~7us of epilogue in the baseline).
"""

import os

import numpy as np

P = 128
B = 8192
D_LAT = 1024
D_EMB = 1024
D = D_LAT + D_EMB  # 2048
E = 8
TOPK = 2
N_CORES = 8
KT = D // P
MT = D // P


# ----------------------------------------------------------------- device ---

_cache = {}


def _ntff_shim():
    """Register the axon NTFF profile hook that the boot skips when
    antenv.axon_hooks is missing (so BASS_TRACE=1 yields exec_time_ns)."""
    import sys
    import types

    if "antenv.axon_hooks" in sys.modules:
        return
    holder = [None]
    mod = types.ModuleType("antenv.axon_hooks")
    mod.set_axon_ntff_profile_hook = lambda h: holder.__setitem__(0, h)
    mod.get_axon_ntff_profile_hook = lambda: holder[0]
    sys.modules["antenv.axon_hooks"] = mod
    try:
        import antenv

        antenv.axon_hooks = mod
        from trn_agent_boot.trn_boot import _ntff_profile_via_ctypes

        mod.set_axon_ntff_profile_hook(
            _ntff_profile_via_ctypes("/opt/axon/libaxon_pjrt.so")
        )
    except Exception:
        pass


def _n_tiles(C):
    """Split C into moving-operand tiles of width 256..512 (float32r needs
    >=256 columns per matmul for full PE rate; PSUM caps a tile at 512).
    Large tiles first so the fetch-bound start phase has dense PE work."""
    assert C >= 512
    k = (C - 256) // 512 if C % 512 else C // 512
    rem = C - 512 * k
    sizes = [512] * k
    if rem == 0:
        pass
    elif rem <= 512:
        sizes.append(rem)
    else:  # 513..767: two tiles, both >= 256
        sizes.extend([rem - 256, 256])
    # smallest tile FIRST (0.26MB first chunk starts the PE sooner and the
    # DVFS ramp taxes 256-col matmuls at half the 512-col cost), 512s in
    # the middle (weight-demand window stays wide), second-smallest last
    # (small tail). Full-ascending is a measured loss (weight window
    # compresses to ~28us and oversubscribes HBM).
    if len(sizes) >= 3:
        sizes = [sizes[-1]] + sizes[:-2] + [sizes[-2]]
    return sizes


def _build(C, dt_name):
    import concourse.mybir as mybir
    from concourse import bacc
    from concourse.bass import ds
    from concourse.tile import TileContext

    dt_in = getattr(mybir.dt, dt_name)
    dt_out = mybir.dt.float32 if dt_name == "float32r" else dt_in
    n_sizes = _n_tiles(C)
    nc = bacc.Bacc(
        "TRN2", target_bir_lowering=False, debug=False, num_devices=N_CORES
    )
    # wp[m, ki, ko, o] = W_e[m*128 + o, ko*128 + ki] — per-m chunks are
    # contiguous (one run per partition) so descriptor gen is cheap.
    wp = nc.dram_tensor("wp", [MT, P, KT, P], dt_in, kind="ExternalInput")
    # toksP[ki, KT*off_n + ko*n_sz + j] = inp[token (off_n + j), ko*128 + ki]
    # — each n-tile (and any k-chunk of it) is one contiguous run per
    # partition, matching the SBUF tile layout exactly.
    toksP = nc.dram_tensor("toksP", [P, KT * C], dt_in, kind="ExternalInput")
    # biasP[mi, mo] = b_e[mo*128 + mi]
    biasP = nc.dram_tensor("biasP", [P, MT], mybir.dt.float32, kind="ExternalInput")
    outT = nc.dram_tensor("outT", [D, C], dt_out, kind="ExternalOutput")

    o_r = outT.ap().rearrange("(mo mi) n -> mi mo n", mi=P)

    with TileContext(nc) as tc:
        with (
            tc.tile_pool(name="w", bufs=1) as w_pool,
            tc.tile_pool(name="tok", bufs=3) as tok_pool,
            tc.tile_pool(name="out", bufs=1) as out_pool,
            tc.tile_pool(name="bias", bufs=1) as b_pool,
            tc.tile_pool(name="ps", bufs=8, space="PSUM") as ps_pool,
        ):
            bias_tile = b_pool.tile([P, MT], mybir.dt.float32)
            tok_tiles = {}
            n_offs = []
            off = 0
            for sz in n_sizes:
                n_offs.append(off)
                off += sz

            def load_toks(n, chunked=False):
                n_sz = n_sizes[n]
                base = KT * n_offs[n]
                t_full = tok_pool.tile(
                    [P, KT * 512], dt_in, tag="tok", name=f"t{n}"
                )
                t_tile = t_full[:, : KT * n_sz]
                if chunked:
                    # 4-k-slice chunks so the first matmuls only wait for
                    # the slices they read
                    step = 4 * n_sz
                    for g in range(0, KT * n_sz, step):
                        nc.sync.dma_start(
                            t_tile[:, ds(g, step)],
                            toksP.ap()[:, ds(base + g, step)],
                        )
                else:
                    # prefetch tiles ride the Act queue BEHIND the weight
                    # chunks: demand-ordered issue keeps the 2MB-per-tile
                    # prefetches out of the BW-crunched startup window
                    # (they stole HBM from w1/w2 and the n0 chunks: ~4us
                    # of early PE stalls)
                    nc.scalar.dma_start(t_tile, toksP.ap()[:, ds(base, KT * n_sz)])
                tok_tiles[n] = t_tile

            w_tiles = [None] * MT

            def load_w(m):
                w_t = w_pool.tile([P, KT, P], dt_in, tag=f"w{m}")
                # weights ride the Activation-HWDGE queues; tokens/outputs
                # ride SP-HWDGE, so the streams don't interleave in one queue
                nc.scalar.dma_start(w_t[:], wp.ap()[m])
                w_tiles[m] = w_t

            # per-m output accumulators: all 5 n-tile drains for row-block m
            # land here, then one [128, C] DMA writes the block out
            out_tiles = [
                out_pool.tile([P, C], dt_out, tag=f"o{m}", name=f"o{m}")
                for m in range(MT)
            ]

            # issue order ~= consumption order: w0 on the Act queue; token
            # tile 0 (k-chunked) leads the SP queue, bias + deep token
            # prefetch behind it; weight chunks stream behind w0.
            # (No PE warmup: the DVFS ramp is ~4.5us of wall time from
            # stream start regardless, and warmup matmuls gated on any DMA
            # just delay the real stream — measured net loss.)
            load_w(0)
            load_toks(0, chunked=True)
            nc.sync.dma_start(bias_tile[:], biasP.ap())
            for m in range(1, MT):
                load_w(m)
            load_toks(1)

            # out halves: the left half DMAs as soon as its last drain lands
            # (during a mid sweep), so the tail only moves the right half
            if len(n_sizes) >= 3:
                n_half = len(n_sizes) - 2
                split_at = n_offs[n_half] + n_sizes[n_half]
            else:
                n_half = -1
                split_at = 0

            for n, n_sz in enumerate(n_sizes):
                if n + 2 < len(n_sizes):
                    load_toks(n + 2)
                t_tile = tok_tiles.pop(n)
                n_off = n_offs[n]
                for m in range(MT):
                    ps_full = ps_pool.tile([P, 512], mybir.dt.float32, tag="ps")
                    ps = ps_full[:, :n_sz]
                    for k in range(KT):
                        nc.tensor.matmul(
                            ps,
                            w_tiles[m][:, k, :],
                            t_tile[:, ds(k * n_sz, n_sz)],
                            start=(k == 0),
                            stop=(k == KT - 1),
                        )
                    nc.vector.tensor_scalar_add(
                        out_tiles[m][:, ds(n_off, n_sz)],
                        ps,
                        bias_tile[:, m : m + 1],
                    )
                    if n == n_half:
                        nc.sync.dma_start(
                            o_r[:, m, ds(0, split_at)],
                            out_tiles[m][:, ds(0, split_at)],
                        )
                    elif n == len(n_sizes) - 1:
                        nc.sync.dma_start(
                            o_r[:, m, ds(split_at, C - split_at)],
                            out_tiles[m][:, ds(split_at, C - split_at)],
                        )
    nc.compile()
    return nc


def _get_program(C, dt_name):
    key = (C, dt_name)
    if key not in _cache:
        _cache[key] = _build(C, dt_name)
    return _cache[key]


# ------------------------------------------------------------------- host ---


def kernel(x, y, W_experts, b_experts, W_gate, b_gate):
    x = np.asarray(x, dtype=np.float32)
    y = np.asarray(y, dtype=np.float32)
    W_experts = np.asarray(W_experts, dtype=np.float32)
    b_experts = np.asarray(b_experts, dtype=np.float32)
    W_gate = np.asarray(W_gate, dtype=np.float32)
    b_gate = np.asarray(b_gate, dtype=np.float32)

    inp = np.concatenate([x, y], axis=1)  # [B, D]

    # ---- routing (host) ----
    logits = inp.astype(np.float64) @ W_gate.T.astype(np.float64) + b_gate
    order = np.argsort(-logits, axis=1, kind="stable")
    top2 = order[:, :TOPK]  # [B, 2]
    v = np.take_along_axis(logits, top2, axis=1)
    v = v - v.max(axis=1, keepdims=True)
    ev = np.exp(v)
    g = (ev / ev.sum(axis=1, keepdims=True)).astype(np.float32)  # [B, 2]

    counts = np.bincount(top2.ravel(), minlength=E)
    C = max(512, int(counts.max()))

    idx_list = []
    wgt_list = []
    for e in range(E):
        m0 = top2[:, 0] == e
        m1 = top2[:, 1] == e
        idx_e = np.concatenate([np.nonzero(m0)[0], np.nonzero(m1)[0]])
        w_e = np.concatenate([g[m0, 0], g[m1, 1]])
        idx_list.append(idx_e)
        wgt_list.append(w_e)

    dt_name = os.environ.get("MOE_DT", "bfloat16")
    if dt_name == "bfloat16":
        import ml_dtypes

        np_in_dt = np.dtype(ml_dtypes.bfloat16)
    else:
        np_in_dt = np.dtype(np.float32)

    n_sizes = _n_tiles(C)
    inpT = np.ascontiguousarray(inp.T)  # [D, B]
    in_maps = []
    for e in range(E):
        toksT = np.zeros((D, C), dtype=np_in_dt)
        toksT[:, : len(idx_list[e])] = inpT[:, idx_list[e]].astype(np_in_dt)
        # toksP[ki, KT*off + ko*sz + j] = toksT[ko*128 + ki, off + j]
        toksP = np.empty((P, KT * C), dtype=np_in_dt)
        off = 0
        for sz in n_sizes:
            blk = toksT[:, off : off + sz].reshape(KT, P, sz)
            toksP[:, KT * off : KT * (off + sz)] = blk.transpose(1, 0, 2).reshape(
                P, KT * sz
            )
            off += sz
        # wp[m, ki, ko, o] = W_e[m*128 + o, ko*128 + ki]
        wp = np.ascontiguousarray(
            W_experts[e].reshape(MT, P, KT, P).transpose(0, 3, 2, 1).astype(np_in_dt)
        )
        biasP = np.ascontiguousarray(b_experts[e].reshape(MT, P).T)
        in_maps.append({"wp": wp, "toksP": toksP, "biasP": biasP})

    # ---- device ----
    if os.environ.get("BASS_TRACE"):
        _ntff_shim()
    from concourse.bass_utils import run_bass_kernel_spmd

    nc = _get_program(C, dt_name)
    res = None
    for attempt in range(3):
        try:
            res = run_bass_kernel_spmd(nc, in_maps, core_ids=list(range(N_CORES)))
            break
        except Exception:
            # the axon-tunneled device occasionally reports a transient
            # NRT_EXEC_UNIT_UNRECOVERABLE; it recovers after a short wait
            if attempt == 2:
                raise
            import time

            time.sleep(20 * (attempt + 1))
            try:
                import jax

                jax.clear_caches()
            except Exception:
                pass
    globals()["_last_res"] = res
    if res.exec_time_ns is not None:
        print(f"HW exec time: {res.exec_time_ns} ns")

    # ---- combine (host) ----
    fused = np.zeros((B, D), dtype=np.float32)
    for e in range(E):
        n_e = len(idx_list[e])
        if n_e == 0:
            continue
        out_rows = np.asarray(
            res.results[e]["outT"][:, :n_e], dtype=np.float32
        ).T  # [n_e, D]
        fused[idx_list[e]] += out_rows * wgt_list[e][:, None]
    return fused
